# revision 1
# baseline (speedup 1.0000x reference)
"""LocalWindowAttention TRN2 kernel.

Full inputs -> full output. Sharding: 8 cores = batch(4) x seq-half(2).
Each core computes 2048 query positions; k/v halos (128 each side) come
from overlapping the per-core x slice, so no collectives are needed.

Math (per core, matching reference):
  qkv = x @ Wqkv + bqkv  (q scaled by 1/sqrt(1024) via the activation)
  banded attention, window 128, block size 128: query tile e attends key
  tiles e-1, e, e+1 with a static band mask |kpos - qpos| <= 128.
  Softmax without max-subtraction (scores are O(0.1)); invalid keys are
  zeroed AFTER exp via a 0/1 band mask, out-of-sequence keys are zeroed
  via a validity indicator carried as a 65th column of v (which also
  yields the softmax denominator through the PV matmul).
  out = attn_out @ Wout + bout

Dtypes: q/k projection in fp8e4m3 DoubleRow (2x PE rate; weights scaled
x50 on host, descaled in the bias activation; position chunks of 512 so
the matmul stream matches the 256-col LDWEIGHTS time). v/out
projections and attention in fp16. Verified numerically: q/k fp8 adds
~6e-3 rel err (scores are O(0.1) so softmax is insensitive); fp8 on v
would add ~3e-2 and is NOT used.

Scheduling: the attention softmax chain (scores -> EXP -> mask -> PV ->
normalize -> transpose) spans four engines with ~1us of latency per
head-pair but only ~0.8us of PE work, so attention-only stretches leave
the PE sparse enough for the HAM clock gate to re-throttle it to
1.2GHz. The emitter therefore interleaves the dense projection matmul
groups (q/k chunks, v tiles, out-projections) 1:1 between attention
pairs as "filler", keeping the PE dense and warm for the whole kernel.
"""

import sys

import numpy as np

for _p in ("/opt/trn_rl_repo",):
    if _p not in sys.path:
        sys.path.insert(0, _p)

import ml_dtypes  # noqa: E402

import concourse.bass as bass  # noqa: E402,F401
import concourse.mybir as mybir  # noqa: E402
import concourse.tile as tile  # noqa: E402
from concourse import bacc  # noqa: E402
from concourse.bass_utils import run_bass_kernel_spmd  # noqa: E402
from concourse.masks import make_identity  # noqa: E402

F32 = mybir.dt.float32
FP16 = mybir.dt.float16
FP8 = mybir.dt.float8e4
NP_FP8 = ml_dtypes.float8_e4m3

B, S, D = 4, 4096, 1024
H, DH, W = 16, 64, 128
N_CORES = 8
S_LOC = 2048            # query positions per core
T_Q = S_LOC // W        # 16 query tiles per core
T_EXT = T_Q + 2         # 18 extended tiles (with halo)
S_EXT = T_EXT * W       # 2304
NQK = 2 * D             # q+k projected features
KC = D // 128           # 8 contraction chunks
HP = H // 2             # 8 head pairs
VCOL = DH + 1           # 64 v dims + indicator column

SIXTH = S_EXT // 6      # 384 positions per x streaming chunk

WSCALE = 50.0           # fp8 weight prescale (host) -> descale in activation
QK_DESCALE = 1.0 / WSCALE
Q_DESCALE = 1.0 / (WSCALE * np.sqrt(D))

# q/k projection position chunks (512-wide so the DoubleRow matmul
# stream time equals its LDWEIGHTS time; q skips the two halo tiles)
Q_CHUNKS = [(128, 512), (640, 512), (1152, 512), (1664, 512)]
K_CHUNKS = [(0, 512), (512, 512), (1024, 512), (1536, 512), (2048, 256)]


def _build_nc():
    nc = bacc.Bacc(
        "TRN2",
        target_bir_lowering=False,
        debug=False,
        num_devices=N_CORES,
    )

    xT_d = nc.dram_tensor("xT", [D, S_EXT], FP16, kind="ExternalInput").ap()
    xT8_d = nc.dram_tensor("xT8", [D, S_EXT], FP8, kind="ExternalInput").ap()
    wqk8_d = nc.dram_tensor("wqk8", [D, NQK], FP8, kind="ExternalInput").ap()
    wv_d = nc.dram_tensor("wv", [D, D], FP16, kind="ExternalInput").ap()
    bqk_d = nc.dram_tensor("bqk", [128, 16], F32, kind="ExternalInput").ap()
    bvb_d = nc.dram_tensor("bvb", [1, D], FP16, kind="ExternalInput").ap()
    wout_d = nc.dram_tensor("wout", [D, D], FP16, kind="ExternalInput").ap()
    boutb_d = nc.dram_tensor("boutb", [1, D], FP16, kind="ExternalInput").ap()
    trimask_d = nc.dram_tensor("trimask", [128, 3 * W], FP16, kind="ExternalInput").ap()
    indp_d = nc.dram_tensor("indp", [T_EXT, 128], F32, kind="ExternalInput").ap()
    out_d = nc.dram_tensor("out", [S_LOC, D], FP16, kind="ExternalOutput").ap()

    with tile.TileContext(nc) as tc:
        _emit(tc, xT_d, xT8_d, wqk8_d, wv_d, bqk_d, bvb_d, wout_d, boutb_d,
              trimask_d, indp_d, out_d)
    nc.compile()
    return nc


def _emit(tc, xT_d, xT8_d, wqk8_d, wv_d, bqk_d, bvb_d, wout_d, boutb_d,
          trimask_d, indp_d, out_d):
    nc = tc.nc

    with (
        tc.tile_pool(name="consts", bufs=1) as consts,
        tc.tile_pool(name="dram", bufs=1, space="DRAM") as dram,
    ):
        # ---- constants (gpsimd ring; bqk first — the first q/k
        # activation needs it) ----
        bqk_sb = consts.tile([128, 16], F32)
        nc.gpsimd.dma_start(bqk_sb[:], bqk_d[:])
        ind_sb = consts.tile([128, T_EXT], F32)
        nc.gpsimd.dma_start(ind_sb[:], indp_d.rearrange("t p -> p t"))
        # the remaining consts are descriptor-heavy broadcasts that
        # starve the first-needed weight/x8 DMAs if issued up front;
        # they are emitted later via emit_late_consts()
        bvb_sb = consts.tile([128, D], FP16)
        trimask_sb = consts.tile([128, 2, 3, W], FP16)
        boutb_sb = consts.tile([128, D], FP16)
        ident_sb = consts.tile([128, 128], FP16)
        make_identity(nc, ident_sb[:])

        def emit_late_consts():
            nc.gpsimd.dma_start(bvb_sb[:], bvb_d.to_broadcast((128, D)))
            for h2 in range(2):
                nc.gpsimd.dma_start(
                    trimask_sb[:, h2],
                    trimask_d.rearrange("p (c w) -> p c w", c=3),
                )
            nc.gpsimd.dma_start(boutb_sb[:], boutb_d.to_broadcast((128, D)))

        with (
            tc.tile_pool(name="stores", bufs=1) as stores,
            tc.tile_pool(name="p2_e", bufs=4) as p2_e,
            tc.tile_pool(name="p2_sm", bufs=6) as p2_sm,
            tc.tile_pool(name="p2_ao", bufs=2) as p2_ao,
            tc.tile_pool(name="p2_out", bufs=2) as p2_out,
            tc.tile_pool(name="p1_ps", bufs=2, space="PSUM") as p1_ps,
            tc.tile_pool(name="ps_s", bufs=2, space="PSUM") as ps_s_pool,
            tc.tile_pool(name="ps_ot", bufs=2, space="PSUM") as ps_ot_pool,
            tc.tile_pool(name="p1_w", bufs=1) as p1_w,
            tc.tile_pool(name="p1_x", bufs=2) as p1_x,
        ):
            # q/k in (feature-pair partitions x positions); v in
            # (positions x head x 65) with the indicator column
            q_store = stores.tile([128, KC, S_EXT], FP16)
            k_store = stores.tile([128, KC, S_EXT], FP16)
            v_store = stores.tile([128, T_EXT, H, VCOL], FP16)
            # fp8 x, fully resident (18KB/partition) so q/k matmul
            # position chunks decouple from the DMA stream
            x8_store = stores.tile([128, KC, S_EXT], FP8)

            wout_sb = p1_w.tile([128, KC, D], FP16)
            w8_sb = p1_w.tile([128, KC, NQK], FP8)
            wv_sb = p1_w.tile([128, KC, D], FP16)

            def emit_x8_dma(sixth):
                tp0 = sixth * SIXTH
                for kc in range(KC):
                    nc.scalar.dma_start(
                        x8_store[:, kc, tp0 : tp0 + SIXTH],
                        xT8_d[kc * 128 : (kc + 1) * 128, tp0 : tp0 + SIXTH],
                    )

            # weights on the sync ring: w8 piece 0 first (first q/k
            # matmuls), wv interleaved early (v tiles need both pieces),
            # wout last (first out-projection is far in)
            w8_piece = lambda p: [
                nc.sync.dma_start(
                    w8_sb[:, kc, p * 512 : (p + 1) * 512],
                    wqk8_d[kc * 128 : (kc + 1) * 128, p * 512 : (p + 1) * 512],
                )
                for kc in range(KC)
            ]
            wv_piece = lambda p: [
                nc.sync.dma_start(
                    wv_sb[:, kc, p * 512 : (p + 1) * 512],
                    wv_d[kc * 128 : (kc + 1) * 128, p * 512 : (p + 1) * 512],
                )
                for kc in range(KC)
            ]
            w8_piece(0)
            wv_piece(0)
            w8_piece(1)
            wv_piece(1)
            w8_piece(2)
            w8_piece(3)
            for kc in range(KC):
                nc.sync.dma_start(wout_sb[:, kc, :], wout_d[kc * 128 : (kc + 1) * 128, :])

            def qk_item(ci, m):
                """One q/k projection matmul group: output feature chunk
                m over position chunk ci (fp8 DoubleRow, 4 accumulating
                matmuls + bias/descale activation). ~0.9us of dense PE."""
                if m < KC:
                    o0, on = Q_CHUNKS[ci]
                    dst, mm = q_store, m
                else:
                    o0, on = K_CHUNKS[ci]
                    dst, mm = k_store, m - KC
                ps = p1_ps.tile([128, 512], F32, tag="ps1", name="ps")
                for kcp in range(KC // 2):
                    nc.tensor.matmul(
                        ps[:, :on],
                        lhsT=w8_sb[:, 2 * kcp : 2 * kcp + 2, m * 128 : (m + 1) * 128],
                        rhs=x8_store[:, 2 * kcp : 2 * kcp + 2, o0 : o0 + on],
                        start=(kcp == 0),
                        stop=(kcp == KC // 2 - 1),
                        perf_mode=mybir.MatmulPerfMode.DoubleRow,
                    )
                nc.scalar.activation(
                    dst[:, mm, o0 : o0 + on],
                    ps[:, :on],
                    mybir.ActivationFunctionType.Identity,
                    bias=bqk_sb[:, m : m + 1],
                    scale=Q_DESCALE if m < KC else QK_DESCALE,
                )

            def qk_chunk_items(ci):
                ms = range(16) if ci < len(Q_CHUNKS) else range(KC, 16)
                return [(lambda ci=ci, m=m: qk_item(ci, m)) for m in ms]

            # per-sixth xT staging for the v projection (pool, 3 bufs)
            xT_tiles = {}

            def v_dma_item(sixth):
                tp0 = sixth * SIXTH
                xT_sb = p1_x.tile([128, KC, SIXTH], FP16, tag="xt", name="xT_sb")
                xT_tiles[sixth] = xT_sb
                for kc in range(KC):
                    nc.scalar.dma_start(
                        xT_sb[:, kc, :],
                        xT_d[kc * 128 : (kc + 1) * 128, tp0 : tp0 + SIXTH],
                    )

            def v_item(t, g):
                """Half of one v tile: 8 accumulating N=512 matmuls +
                bias add. ~1.7us of dense PE. g==1 also finalizes the
                tile (indicator column; boundary-tile zeroing)."""
                sixth, tt = t // 3, t % 3
                xT_sb = xT_tiles[sixth]
                ps = p1_ps.tile([128, 512], F32, tag="ps1", name="ps")
                for kc in range(KC):
                    nc.tensor.matmul(
                        ps[:],
                        lhsT=xT_sb[:, kc, tt * 128 : (tt + 1) * 128],
                        rhs=wv_sb[:, kc, g * 512 : (g + 1) * 512],
                        start=(kc == 0),
                        stop=(kc == KC - 1),
                    )
                nc.vector.tensor_add(
                    v_store[:, t, g * 8 : (g + 1) * 8, 0:DH],
                    ps.rearrange("p (h d) -> p h d", d=DH),
                    bvb_sb[:, g * 512 : (g + 1) * 512].rearrange(
                        "p (h d) -> p h d", d=DH
                    ),
                )
                if g == 1 and t in (0, T_EXT - 1):
                    # only the two halo tiles can be out-of-sequence
                    # (the indicator column itself is set by one
                    # upfront memset over all tiles)
                    nc.vector.tensor_scalar_mul(
                        v_store[:, t], v_store[:, t], ind_sb[:, t : t + 1]
                    )

            def v_sixth_items(sixth):
                items = [lambda sixth=sixth: v_dma_item(sixth)]
                for tt in range(3):
                    t = sixth * 3 + tt
                    for g in range(2):
                        items.append(lambda t=t, g=g: v_item(t, g))
                return items

            def out_proj_item(e, g, aoT_sb):
                """Out-projection half for query tile e: 8 accumulating
                N=512 matmuls + bias + store. ~1.8us of dense PE."""
                ps_f = ps_ot_pool.tile([128, 512], F32, tag="ot", name="ps_f")
                for a in range(HP):
                    nc.tensor.matmul(
                        ps_f[:],
                        lhsT=aoT_sb[:, a, :],
                        rhs=wout_sb[:, a, g * 512 : (g + 1) * 512],
                        start=(a == 0),
                        stop=(a == HP - 1),
                    )
                fo = p2_out.tile([128, 512], FP16, tag="fo", name="fo")
                nc.vector.tensor_add(
                    fo[:], ps_f[:], boutb_sb[:, g * 512 : (g + 1) * 512]
                )
                eng = nc.sync if g == 0 else nc.gpsimd
                eng.dma_start(
                    out_d[(e - 1) * 128 : e * 128, g * 512 : (g + 1) * 512],
                    fo[:],
                )

            def emit_pair(e, a, aoT_sb):
                """Softmax chain for head-pair a of query tile e."""
                # scoresT per head; c-outer/h2-inner alternates PE row
                # groups (tile_position (0,0)/(64,0)) between adjacent
                # matmuls so LDWEIGHTS can pull ahead and the two
                # 64-contraction matmuls can overlap in the array
                ps_s = ps_s_pool.tile([128, 2, 4, W], F32, tag="ps_s", name="ps_s")
                for c in range(3):
                    t = e - 1 + c
                    for h2 in range(2):
                        pr = slice(64 * h2, 64 * h2 + 64)
                        nc.tensor.matmul(
                            ps_s[:, h2, c, :],
                            lhsT=k_store[pr, a, t * 128 : (t + 1) * 128],
                            rhs=q_store[pr, a, e * 128 : (e + 1) * 128],
                            start=True,
                            stop=True,
                        )
                e_sb = p2_e.tile([128, 2, 3, W], FP16, name="e_sb")
                nc.scalar.activation(
                    e_sb[:],
                    ps_s[:, :, 0:3, :],
                    mybir.ActivationFunctionType.Exp,
                )
                # band mask: only the two triangular chunks (c=1 is
                # fully in-band); DVE — GpSimd is 3.5x slower and was
                # sitting on the critical path
                nc.vector.tensor_mul(
                    e_sb[:, :, 0:3:2, :],
                    e_sb[:, :, 0:3:2, :],
                    trimask_sb[:, :, 0:3:2, :],
                )

                # both heads' PV outputs share ONE PSUM bank
                # ([128, 2, 65] = 520B) — halves the ot-pool allocations
                # per pair, which throttle the pipeline
                no_sb = p2_sm.tile([128, 2, DH], FP16, tag="no", name="no_sb")
                ps_o = ps_ot_pool.tile([128, 2, VCOL], F32, tag="ot", name="ps_o")
                for h2 in range(2):
                    for c in range(3):
                        nc.tensor.matmul(
                            ps_o[:, h2, :],
                            lhsT=e_sb[:, h2, c, :],
                            rhs=v_store[:, e - 1 + c, 2 * a + h2, :],
                            start=(c == 0),
                            stop=(c == 2),
                        )
                rcp = p2_sm.tile([128, 2], F32, tag="rcp", name="rcp")
                nc.vector.reciprocal(rcp[:], ps_o[:, :, DH])
                for h2 in range(2):
                    nc.vector.tensor_scalar_mul(
                        no_sb[:, h2, :], ps_o[:, h2, 0:DH], rcp[:, h2 : h2 + 1]
                    )
                ps_t = ps_ot_pool.tile([128, 512], F32, tag="ot", name="ps_t")
                ps_t16 = ps_t.bitcast(FP16)[:, :128]
                nc.tensor.transpose(
                    ps_t16[:], no_sb.rearrange("p a d -> p (a d)"), ident_sb[:]
                )
                if a % 2 == 0:
                    nc.scalar.copy(aoT_sb[:, a, :], ps_t16[:])
                else:
                    nc.vector.tensor_copy(aoT_sb[:, a, :], ps_t16[:])

            # ---- interleaved emission: attention pairs alternate with
            # dense projection "filler" items ----
            def run_attention(tiles, filler, prev_state):
                """Emit attention for `tiles`, interleaving one filler
                item after each head-pair. prev_state carries the
                previous tile's aoT so its out-projection can be
                emitted (as filler-priority work) during this tile."""
                prev = prev_state
                for e in tiles:
                    aoT_sb = p2_ao.tile([128, HP, 128], FP16, name="aoT")
                    for a in range(HP):
                        emit_pair(e, a, aoT_sb)
                        if prev is not None and a == 2:
                            out_proj_item(e - 1, 0, prev)
                        elif prev is not None and a == 5:
                            out_proj_item(e - 1, 1, prev)
                        elif filler:
                            filler.pop(0)()
                    prev = aoT_sb
                return prev

            # the v indicator column is constant 1.0 across all tiles
            # (the projection writes cols 0:DH only — disjoint), so one
            # upfront memset covers the whole kernel
            nc.vector.memset(v_store[:, :, :, DH : DH + 1], 1.0)

            # phase 0: projections only (nothing to interleave yet)
            emit_x8_dma(0)
            emit_x8_dma(1)
            for it in qk_chunk_items(0):
                it()
            emit_late_consts()
            emit_x8_dma(2)
            # interleave the first v tiles into the second q/k chunk so
            # the v pipeline starts as soon as wv/xT land
            f = v_sixth_items(0) + v_sixth_items(1)
            for it in qk_chunk_items(1):
                it()
                if f:
                    f.pop(0)()
            emit_x8_dma(3)
            emit_x8_dma(4)
            for it in f:
                it()

            prev = None
            filler = qk_chunk_items(2) + v_sixth_items(2)
            prev = run_attention((1, 2, 3), filler, prev)
            emit_x8_dma(5)
            filler = filler + qk_chunk_items(3) + v_sixth_items(3)
            prev = run_attention((4, 5, 6), filler, prev)
            filler = filler + qk_chunk_items(4) + v_sixth_items(4)
            prev = run_attention((7, 8, 9), filler, prev)
            filler = filler + v_sixth_items(5)
            prev = run_attention((10, 11, 12), filler, prev)
            prev = run_attention((13, 14, 15, 16), filler, prev)
            for it in filler:
                it()
            out_proj_item(16, 0, prev)
            out_proj_item(16, 1, prev)


_NC_CACHE = None


def _get_nc():
    global _NC_CACHE
    if _NC_CACHE is None:
        _NC_CACHE = _build_nc()
    return _NC_CACHE


def _host_inputs(x, Wqkv, bqkv, Wout, bout):
    """Build the 8 per-core input maps."""
    x = np.asarray(x, dtype=np.float32)
    Wqkv = np.asarray(Wqkv, dtype=np.float32)
    bqkv = np.asarray(bqkv, dtype=np.float32)
    Wout = np.asarray(Wout, dtype=np.float32)
    bout = np.asarray(bout, dtype=np.float32)

    scale = np.float32(1.0 / np.sqrt(D))
    bs = bqkv.copy()
    bs[:D] *= scale

    # q/k weights: prescale x50 for fp8 range (descaled in the activation)
    wqk8 = np.clip(Wqkv[:, :NQK] * WSCALE, -240.0, 240.0).astype(NP_FP8)
    wv16 = Wqkv[:, NQK:].astype(np.float16)

    bqk = np.ascontiguousarray(bs[:NQK].reshape(16, 128).T)  # (128, 16)
    bvb = bs[NQK:].reshape(1, D)
    boutb = bout.reshape(1, D)

    # band mask in (j_within_chunk, chunk, i) layout flattened to (128, 384)
    jc = np.arange(128)[:, None]
    i = np.arange(128)[None, :]
    tm = np.ones((128, 3, 128), dtype=np.float32)
    tm[:, 0] = (jc >= i).astype(np.float32)
    tm[:, 2] = (jc <= i).astype(np.float32)
    trimask = tm.reshape(128, 3 * W).astype(np.float16)

    in_maps = []
    for core in range(N_CORES):
        b, half = core // 2, core % 2
        s0 = half * S_LOC
        lo, hi = s0 - W, s0 + S_LOC + W
        xp = np.zeros((S_EXT, D), dtype=np.float32)
        src_lo, src_hi = max(lo, 0), min(hi, S)
        xp[src_lo - lo : src_hi - lo] = x[b, src_lo:src_hi]
        xpT = np.ascontiguousarray(xp.T)
        xT = xpT.astype(np.float16)
        xT8 = np.clip(xpT, -240.0, 240.0).astype(NP_FP8)

        valid = np.ones(S_EXT, dtype=np.float32)
        if lo < 0:
            valid[: -lo] = 0.0
        if hi > S:
            valid[S - hi :] = 0.0
        indp = np.ascontiguousarray(valid.reshape(T_EXT, 128))

        in_maps.append(
            {
                "xT": xT,
                "xT8": xT8,
                "wqk8": wqk8,
                "wv": wv16,
                "bqk": bqk,
                "bvb": bvb.astype(np.float16),
                "wout": Wout.astype(np.float16),
                "boutb": boutb.astype(np.float16),
                "trimask": trimask,
                "indp": indp,
            }
        )
    return in_maps


def kernel(x, Wqkv, bqkv, Wout, bout, _trace=False, _trace_cores=None):
    in_maps = _host_inputs(x, Wqkv, bqkv, Wout, bout)
    nc = _get_nc()
    res = run_bass_kernel_spmd(
        nc,
        in_maps,
        list(range(N_CORES)),
        trace=_trace,
        trace_cores=_trace_cores,
    )
    out = np.empty((B, S, D), dtype=np.float32)
    for core in range(N_CORES):
        b, half = core // 2, core % 2
        s0 = half * S_LOC
        out[b, s0 : s0 + S_LOC] = np.asarray(
            res.results[core]["out"], dtype=np.float32
        )
    if _trace:
        return out, res
    return out



# revision 7
# speedup vs baseline: 1.0377x; 1.0377x over previous
"""LocalWindowAttention TRN2 kernel.

Full inputs -> full output. Sharding: 8 cores = batch(4) x seq-half(2).
Each core computes 2048 query positions; k/v halos (128 each side) come
from overlapping the per-core x slice, so no collectives are needed.

Math (per core, matching reference):
  qkv = x @ Wqkv + bqkv  (q scaled by 1/sqrt(1024) via the activation)
  banded attention, window 128, block size 128: query tile e attends key
  tiles e-1, e, e+1 with a static band mask |kpos - qpos| <= 128.
  Softmax without max-subtraction (scores are O(0.1)); invalid keys are
  zeroed AFTER exp via a 0/1 band mask, out-of-sequence keys are zeroed
  via a validity indicator carried as a 65th column of v (which also
  yields the softmax denominator through the PV matmul).
  out = attn_out @ Wout + bout

Dtypes: q/k projection in fp8e4m3 DoubleRow (2x PE rate; weights scaled
x50 on host, descaled in the bias activation; position chunks of 512 so
the matmul stream matches the 256-col LDWEIGHTS time). v/out
projections and attention in fp16. Verified numerically: q/k fp8 adds
~6e-3 rel err (scores are O(0.1) so softmax is insensitive); fp8 on v
would add ~3e-2 and is NOT used.

Scheduling: the attention softmax chain (scores -> EXP -> mask -> PV ->
normalize -> transpose) spans four engines with ~1us of latency per
head-pair but only ~0.8us of PE work, so attention-only stretches leave
the PE sparse enough for the HAM clock gate to re-throttle it to
1.2GHz. The emitter therefore interleaves the dense projection matmul
groups (q/k chunks, v tiles, out-projections) 1:1 between attention
pairs as "filler", keeping the PE dense and warm for the whole kernel.
"""

import sys

import numpy as np

for _p in ("/opt/trn_rl_repo",):
    if _p not in sys.path:
        sys.path.insert(0, _p)

import ml_dtypes  # noqa: E402

import concourse.bass as bass  # noqa: E402,F401
import concourse.mybir as mybir  # noqa: E402
import concourse.tile as tile  # noqa: E402
from concourse import bacc  # noqa: E402
from concourse.bass_utils import run_bass_kernel_spmd  # noqa: E402
from concourse.masks import make_identity  # noqa: E402

F32 = mybir.dt.float32
FP16 = mybir.dt.float16
FP8 = mybir.dt.float8e4
NP_FP8 = ml_dtypes.float8_e4m3

B, S, D = 4, 4096, 1024
H, DH, W = 16, 64, 128
N_CORES = 8
S_LOC = 2048            # query positions per core
T_Q = S_LOC // W        # 16 query tiles per core
T_EXT = T_Q + 2         # 18 extended tiles (with halo)
S_EXT = T_EXT * W       # 2304
NQK = 2 * D             # q+k projected features
KC = D // 128           # 8 contraction chunks
HP = H // 2             # 8 head pairs
VCOL = DH + 1           # 64 v dims + indicator column

SIXTH = S_EXT // 6      # 384 positions per x streaming chunk

WSCALE = 50.0           # fp8 weight prescale (host) -> descale in activation
QK_DESCALE = 1.0 / WSCALE
Q_DESCALE = 1.0 / (WSCALE * np.sqrt(D))

# q/k projection position chunks (512-wide so the DoubleRow matmul
# stream time equals its LDWEIGHTS time; q skips the two halo tiles)
Q_CHUNKS = [(128, 512), (640, 512), (1152, 512), (1664, 512)]
K_CHUNKS = [(0, 512), (512, 512), (1024, 512), (1536, 512), (2048, 256)]


def _build_nc():
    nc = bacc.Bacc(
        "TRN2",
        target_bir_lowering=False,
        debug=False,
        num_devices=N_CORES,
    )

    xT_d = nc.dram_tensor("xT", [D, S_EXT], FP16, kind="ExternalInput").ap()
    xT8_d = nc.dram_tensor("xT8", [D, S_EXT], FP8, kind="ExternalInput").ap()
    wqk8_d = nc.dram_tensor("wqk8", [D, NQK], FP8, kind="ExternalInput").ap()
    wv_d = nc.dram_tensor("wv", [D, D], FP16, kind="ExternalInput").ap()
    bqk_d = nc.dram_tensor("bqk", [128, 16], F32, kind="ExternalInput").ap()
    bvb_d = nc.dram_tensor("bvb", [1, D], FP16, kind="ExternalInput").ap()
    wout_d = nc.dram_tensor("wout", [D, D], FP16, kind="ExternalInput").ap()
    boutb_d = nc.dram_tensor("boutb", [1, D], FP16, kind="ExternalInput").ap()
    trimask_d = nc.dram_tensor("trimask", [128, 3 * W], FP16, kind="ExternalInput").ap()
    indp_d = nc.dram_tensor("indp", [T_EXT, 128], F32, kind="ExternalInput").ap()
    out_d = nc.dram_tensor("out", [S_LOC, D], FP16, kind="ExternalOutput").ap()

    with tile.TileContext(nc) as tc:
        _emit(tc, xT_d, xT8_d, wqk8_d, wv_d, bqk_d, bvb_d, wout_d, boutb_d,
              trimask_d, indp_d, out_d)
    nc.compile()
    return nc


def _emit(tc, xT_d, xT8_d, wqk8_d, wv_d, bqk_d, bvb_d, wout_d, boutb_d,
          trimask_d, indp_d, out_d):
    nc = tc.nc

    with (
        tc.tile_pool(name="consts", bufs=1) as consts,
        tc.tile_pool(name="dram", bufs=1, space="DRAM") as dram,
    ):
        # ---- constants (gpsimd ring; bqk first — the first q/k
        # activation needs it) ----
        bqk_sb = consts.tile([128, 16], F32)
        nc.gpsimd.dma_start(bqk_sb[:], bqk_d[:])
        ind_sb = consts.tile([128, T_EXT], F32)
        nc.gpsimd.dma_start(ind_sb[:], indp_d.rearrange("t p -> p t"))
        # the remaining consts are descriptor-heavy broadcasts that
        # starve the first-needed weight/x8 DMAs if issued up front;
        # they are emitted later via emit_late_consts()
        bvb_sb = consts.tile([128, D], FP16)
        trimask_sb = consts.tile([128, 2, 3, W], FP16)
        boutb_sb = consts.tile([128, D], FP16)
        ident_sb = consts.tile([128, 128], FP16)
        make_identity(nc, ident_sb[:])

        def emit_late_consts():
            nc.gpsimd.dma_start(bvb_sb[:], bvb_d.to_broadcast((128, D)))
            for h2 in range(2):
                nc.gpsimd.dma_start(
                    trimask_sb[:, h2],
                    trimask_d.rearrange("p (c w) -> p c w", c=3),
                )
            nc.gpsimd.dma_start(boutb_sb[:], boutb_d.to_broadcast((128, D)))

        with (
            tc.tile_pool(name="stores", bufs=1) as stores,
            tc.tile_pool(name="p2_e", bufs=4) as p2_e,
            tc.tile_pool(name="p2_sm", bufs=6) as p2_sm,
            tc.tile_pool(name="p2_ao", bufs=4) as p2_ao,
            tc.tile_pool(name="p2_out", bufs=2) as p2_out,
            tc.tile_pool(name="p1_ps", bufs=2, space="PSUM") as p1_ps,
            tc.tile_pool(name="ps_s", bufs=2, space="PSUM") as ps_s_pool,
            tc.tile_pool(name="ps_ot", bufs=2, space="PSUM") as ps_ot_pool,
            tc.tile_pool(name="p1_w", bufs=1) as p1_w,
            tc.tile_pool(name="p1_x", bufs=2) as p1_x,
        ):
            # q/k in (feature-pair partitions x positions); v in
            # (positions x head x 65) with the indicator column
            # q skips the two halo tiles: columns are positions 128..2176
            # of the extended axis, stored at offset-128
            q_store = stores.tile([128, KC, S_LOC], FP16)
            k_store = stores.tile([128, KC, S_EXT], FP16)
            v_store = stores.tile([128, T_EXT, H, VCOL], FP16)
            # fp8 x, fully resident (18KB/partition) so q/k matmul
            # position chunks decouple from the DMA stream
            x8_store = stores.tile([128, KC, S_EXT], FP8)

            wout_sb = p1_w.tile([128, KC, D], FP16)
            w8_sb = p1_w.tile([128, KC, NQK], FP8)
            wv_sb = p1_w.tile([128, KC, D], FP16)

            def emit_x8_dma(sixth):
                tp0 = sixth * SIXTH
                for kc in range(KC):
                    nc.scalar.dma_start(
                        x8_store[:, kc, tp0 : tp0 + SIXTH],
                        xT8_d[kc * 128 : (kc + 1) * 128, tp0 : tp0 + SIXTH],
                    )

            # weights on the sync ring: w8 piece 0 first (first q/k
            # matmuls), wv interleaved early (v tiles need both pieces),
            # wout last (first out-projection is far in)
            w8_piece = lambda p: [
                nc.sync.dma_start(
                    w8_sb[:, kc, p * 512 : (p + 1) * 512],
                    wqk8_d[kc * 128 : (kc + 1) * 128, p * 512 : (p + 1) * 512],
                )
                for kc in range(KC)
            ]
            wv_piece = lambda p: [
                nc.sync.dma_start(
                    wv_sb[:, kc, p * 512 : (p + 1) * 512],
                    wv_d[kc * 128 : (kc + 1) * 128, p * 512 : (p + 1) * 512],
                )
                for kc in range(KC)
            ]
            w8_piece(0)
            wv_piece(0)
            w8_piece(1)
            wv_piece(1)
            w8_piece(2)
            w8_piece(3)
            for kc in range(KC):
                nc.sync.dma_start(wout_sb[:, kc, :], wout_d[kc * 128 : (kc + 1) * 128, :])

            def qk_item(ci, m):
                """One q/k projection matmul group: output feature chunk
                m over position chunk ci (fp8 DoubleRow, 4 accumulating
                matmuls + bias/descale activation). ~0.9us of dense PE."""
                if m < KC:
                    o0, on = Q_CHUNKS[ci]
                    dst, mm = q_store, m
                else:
                    o0, on = K_CHUNKS[ci]
                    dst, mm = k_store, m - KC
                ps = p1_ps.tile([128, 512], F32, tag="ps1", name="ps")
                for kcp in range(KC // 2):
                    nc.tensor.matmul(
                        ps[:, :on],
                        lhsT=w8_sb[:, 2 * kcp : 2 * kcp + 2, m * 128 : (m + 1) * 128],
                        rhs=x8_store[:, 2 * kcp : 2 * kcp + 2, o0 : o0 + on],
                        start=(kcp == 0),
                        stop=(kcp == KC // 2 - 1),
                        perf_mode=mybir.MatmulPerfMode.DoubleRow,
                    )
                d0 = o0 - 128 if m < KC else o0  # q_store is offset by the halo
                nc.scalar.activation(
                    dst[:, mm, d0 : d0 + on],
                    ps[:, :on],
                    mybir.ActivationFunctionType.Identity,
                    bias=bqk_sb[:, m : m + 1],
                    scale=Q_DESCALE if m < KC else QK_DESCALE,
                )

            def qk_chunk_items(ci):
                ms = range(16) if ci < len(Q_CHUNKS) else range(KC, 16)
                return [(lambda ci=ci, m=m: qk_item(ci, m)) for m in ms]

            # per-sixth xT staging for the v projection (pool, 3 bufs)
            xT_tiles = {}

            def v_dma_item(sixth):
                tp0 = sixth * SIXTH
                xT_sb = p1_x.tile([128, KC, SIXTH], FP16, tag="xt", name="xT_sb")
                xT_tiles[sixth] = xT_sb
                for kc in range(KC):
                    nc.scalar.dma_start(
                        xT_sb[:, kc, :],
                        xT_d[kc * 128 : (kc + 1) * 128, tp0 : tp0 + SIXTH],
                    )

            def v_item(t, g):
                """Half of one v tile: 8 accumulating N=512 matmuls +
                bias add. ~1.7us of dense PE. g==1 also finalizes the
                tile (indicator column; boundary-tile zeroing)."""
                sixth, tt = t // 3, t % 3
                xT_sb = xT_tiles[sixth]
                ps = p1_ps.tile([128, 512], F32, tag="ps1", name="ps")
                for kc in range(KC):
                    nc.tensor.matmul(
                        ps[:],
                        lhsT=xT_sb[:, kc, tt * 128 : (tt + 1) * 128],
                        rhs=wv_sb[:, kc, g * 512 : (g + 1) * 512],
                        start=(kc == 0),
                        stop=(kc == KC - 1),
                    )
                nc.vector.tensor_add(
                    v_store[:, t, g * 8 : (g + 1) * 8, 0:DH],
                    ps.rearrange("p (h d) -> p h d", d=DH),
                    bvb_sb[:, g * 512 : (g + 1) * 512].rearrange(
                        "p (h d) -> p h d", d=DH
                    ),
                )
                if g == 1 and t in (0, T_EXT - 1):
                    # only the two halo tiles can be out-of-sequence
                    # (the indicator column itself is set by one
                    # upfront memset over all tiles)
                    nc.vector.tensor_scalar_mul(
                        v_store[:, t], v_store[:, t], ind_sb[:, t : t + 1]
                    )

            def v_sixth_items(sixth):
                items = [lambda sixth=sixth: v_dma_item(sixth)]
                for tt in range(3):
                    t = sixth * 3 + tt
                    for g in range(2):
                        items.append(lambda t=t, g=g: v_item(t, g))
                return items

            def out_proj_item(e, g, aoT_sb):
                """Out-projection half for query tile e: 8 accumulating
                N=512 matmuls + bias + store. ~1.8us of dense PE."""
                ps_f = ps_ot_pool.tile([128, 512], F32, tag="ot", name="ps_f")
                for a in range(HP):
                    nc.tensor.matmul(
                        ps_f[:],
                        lhsT=aoT_sb[:, a, :],
                        rhs=wout_sb[:, a, g * 512 : (g + 1) * 512],
                        start=(a == 0),
                        stop=(a == HP - 1),
                    )
                fo = p2_out.tile([128, 512], FP16, tag="fo", name="fo")
                nc.vector.tensor_add(
                    fo[:], ps_f[:], boutb_sb[:, g * 512 : (g + 1) * 512]
                )
                eng = nc.sync if g == 0 else nc.gpsimd
                eng.dma_start(
                    out_d[(e - 1) * 128 : e * 128, g * 512 : (g + 1) * 512],
                    fo[:],
                )

            def emit_pair(e, a, aoT_sb):
                """Softmax chain for head-pair a of query tile e."""
                # scoresT per head; c-outer/h2-inner alternates PE row
                # groups (tile_position (0,0)/(64,0)) between adjacent
                # matmuls so LDWEIGHTS can pull ahead and the two
                # 64-contraction matmuls can overlap in the array
                ps_s = ps_s_pool.tile([128, 2, 4, W], F32, tag="ps_s", name="ps_s")
                for c in range(3):
                    t = e - 1 + c
                    for h2 in range(2):
                        pr = slice(64 * h2, 64 * h2 + 64)
                        nc.tensor.matmul(
                            ps_s[:, h2, c, :],
                            lhsT=k_store[pr, a, t * 128 : (t + 1) * 128],
                            rhs=q_store[pr, a, (e - 1) * 128 : e * 128],
                            start=True,
                            stop=True,
                        )
                e_sb = p2_e.tile([128, 2, 3, W], FP16, name="e_sb")
                nc.scalar.activation(
                    e_sb[:],
                    ps_s[:, :, 0:3, :],
                    mybir.ActivationFunctionType.Exp,
                )
                # band mask: only the two triangular chunks (c=1 is
                # fully in-band); DVE — GpSimd is 3.5x slower and was
                # sitting on the critical path
                nc.vector.tensor_mul(
                    e_sb[:, :, 0:3:2, :],
                    e_sb[:, :, 0:3:2, :],
                    trimask_sb[:, :, 0:3:2, :],
                )

                # both heads' PV outputs share ONE PSUM bank
                # ([128, 2, 65] = 520B) — halves the ot-pool allocations
                # per pair, which throttle the pipeline
                no_sb = p2_sm.tile([128, 2, DH], FP16, tag="no", name="no_sb")
                ps_o = ps_ot_pool.tile([128, 2, VCOL], F32, tag="ot", name="ps_o")
                for h2 in range(2):
                    for c in range(3):
                        nc.tensor.matmul(
                            ps_o[:, h2, :],
                            lhsT=e_sb[:, h2, c, :],
                            rhs=v_store[:, e - 1 + c, 2 * a + h2, :],
                            start=(c == 0),
                            stop=(c == 2),
                        )
                rcp = p2_sm.tile([128, 2], F32, tag="rcp", name="rcp")
                nc.vector.reciprocal(rcp[:], ps_o[:, :, DH])
                for h2 in range(2):
                    nc.vector.tensor_scalar_mul(
                        no_sb[:, h2, :], ps_o[:, h2, 0:DH], rcp[:, h2 : h2 + 1]
                    )
                ps_t = ps_ot_pool.tile([128, 512], F32, tag="ot", name="ps_t")
                ps_t16 = ps_t.bitcast(FP16)[:, :128]
                nc.tensor.transpose(
                    ps_t16[:], no_sb.rearrange("p a d -> p (a d)"), ident_sb[:]
                )
                if a % 2 == 0:
                    nc.scalar.copy(aoT_sb[:, a, :], ps_t16[:])
                else:
                    nc.vector.tensor_copy(aoT_sb[:, a, :], ps_t16[:])

            # the v indicator column is constant 1.0 across all tiles
            # (the projection writes cols 0:DH only — disjoint), so one
            # upfront memset covers the whole kernel
            nc.vector.memset(v_store[:, :, :, DH : DH + 1], 1.0)

            # phase 0: projections only (nothing to interleave yet)
            emit_x8_dma(0)
            emit_x8_dma(1)
            for it in qk_chunk_items(0):
                it()
            emit_late_consts()
            emit_x8_dma(2)
            # interleave the first v tiles into the second q/k chunk so
            # the v pipeline starts as soon as wv/xT land
            f = v_sixth_items(0) + v_sixth_items(1)
            for it in qk_chunk_items(1):
                it()
                if f:
                    f.pop(0)()
            emit_x8_dma(3)
            emit_x8_dma(4)
            for it in f:
                it()

            # ---- interleaved emission: uniform dense-work pacing.
            # The remaining projection items (deadline-bearing, proj_q)
            # and the out-projections (freely deferrable, out_q; each
            # tile's out-proj is enqueued two tiles after its attention)
            # are drained at a constant rate across all 128 pair slots
            # so the PE never goes sparse enough for the HAM clock gate
            # to re-throttle it — previously the fillers ran dry around
            # tile 11 and the tail ran at 1.2GHz two-thirds of the time.
            # v items lead their qk phase-mates: their deadlines (the
            # attention tiles that read v_store) are tighter. All proj
            # deadlines are met at a drain pace of 68 items over the
            # first ~100 pair slots (checked against each refill's
            # first-use tile with >=3 slots of margin).
            proj_q = v_sixth_items(2) + qk_chunk_items(2)
            out_q = []
            out_emitted = 0
            ao_live = {}
            acc_p = 0.0
            acc_o = 0.0
            pace_p = 68.0 / 100.0
            pace_o = 2.0 / HP  # matches the enqueue rate: backlog <= 1 tile
            for e in range(1, T_Q + 1):
                if e == 4:
                    emit_x8_dma(5)
                    proj_q += v_sixth_items(3) + qk_chunk_items(3)
                elif e == 7:
                    proj_q += v_sixth_items(4) + qk_chunk_items(4)
                elif e == 9:
                    proj_q += v_sixth_items(5)
                eo = e - 2
                if eo in ao_live:
                    aoT_prev = ao_live.pop(eo)
                    out_q.append(
                        lambda eo=eo, t=aoT_prev: out_proj_item(eo, 0, t)
                    )
                    out_q.append(
                        lambda eo=eo, t=aoT_prev: out_proj_item(eo, 1, t)
                    )
                # the p2_ao buf reused by tile e held tile e-4's aoT: its
                # out-projection must be EMITTED before this allocation
                # or the pool dependency cycles (deadlock in scheduling)
                while out_emitted < 2 * max(0, e - 4) and out_q:
                    out_q.pop(0)()
                    out_emitted += 1
                aoT_sb = p2_ao.tile([128, HP, 128], FP16, name="aoT")
                ao_live[e] = aoT_sb
                for a in range(HP):
                    emit_pair(e, a, aoT_sb)
                    acc_p += pace_p
                    while acc_p >= 1.0 and proj_q:
                        acc_p -= 1.0
                        proj_q.pop(0)()
                    acc_o += pace_o
                    while acc_o >= 1.0 and out_q:
                        acc_o -= 1.0
                        out_q.pop(0)()
                        out_emitted += 1
            for it in proj_q + out_q:
                it()
            for eo in sorted(ao_live):
                out_proj_item(eo, 0, ao_live[eo])
                out_proj_item(eo, 1, ao_live[eo])


_NC_CACHE = None


def _get_nc():
    global _NC_CACHE
    if _NC_CACHE is None:
        _NC_CACHE = _build_nc()
    return _NC_CACHE


def _host_inputs(x, Wqkv, bqkv, Wout, bout):
    """Build the 8 per-core input maps."""
    x = np.asarray(x, dtype=np.float32)
    Wqkv = np.asarray(Wqkv, dtype=np.float32)
    bqkv = np.asarray(bqkv, dtype=np.float32)
    Wout = np.asarray(Wout, dtype=np.float32)
    bout = np.asarray(bout, dtype=np.float32)

    scale = np.float32(1.0 / np.sqrt(D))
    bs = bqkv.copy()
    bs[:D] *= scale

    # q/k weights: prescale x50 for fp8 range (descaled in the activation)
    wqk8 = np.clip(Wqkv[:, :NQK] * WSCALE, -240.0, 240.0).astype(NP_FP8)
    wv16 = Wqkv[:, NQK:].astype(np.float16)

    bqk = np.ascontiguousarray(bs[:NQK].reshape(16, 128).T)  # (128, 16)
    bvb = bs[NQK:].reshape(1, D)
    boutb = bout.reshape(1, D)

    # band mask in (j_within_chunk, chunk, i) layout flattened to (128, 384)
    jc = np.arange(128)[:, None]
    i = np.arange(128)[None, :]
    tm = np.ones((128, 3, 128), dtype=np.float32)
    tm[:, 0] = (jc >= i).astype(np.float32)
    tm[:, 2] = (jc <= i).astype(np.float32)
    trimask = tm.reshape(128, 3 * W).astype(np.float16)

    in_maps = []
    for core in range(N_CORES):
        b, half = core // 2, core % 2
        s0 = half * S_LOC
        lo, hi = s0 - W, s0 + S_LOC + W
        xp = np.zeros((S_EXT, D), dtype=np.float32)
        src_lo, src_hi = max(lo, 0), min(hi, S)
        xp[src_lo - lo : src_hi - lo] = x[b, src_lo:src_hi]
        xpT = np.ascontiguousarray(xp.T)
        xT = xpT.astype(np.float16)
        xT8 = np.clip(xpT, -240.0, 240.0).astype(NP_FP8)

        valid = np.ones(S_EXT, dtype=np.float32)
        if lo < 0:
            valid[: -lo] = 0.0
        if hi > S:
            valid[S - hi :] = 0.0
        indp = np.ascontiguousarray(valid.reshape(T_EXT, 128))

        in_maps.append(
            {
                "xT": xT,
                "xT8": xT8,
                "wqk8": wqk8,
                "wv": wv16,
                "bqk": bqk,
                "bvb": bvb.astype(np.float16),
                "wout": Wout.astype(np.float16),
                "boutb": boutb.astype(np.float16),
                "trimask": trimask,
                "indp": indp,
            }
        )
    return in_maps


def kernel(x, Wqkv, bqkv, Wout, bout, _trace=False, _trace_cores=None):
    in_maps = _host_inputs(x, Wqkv, bqkv, Wout, bout)
    nc = _get_nc()
    res = run_bass_kernel_spmd(
        nc,
        in_maps,
        list(range(N_CORES)),
        trace=_trace,
        trace_cores=_trace_cores,
    )
    out = np.empty((B, S, D), dtype=np.float32)
    for core in range(N_CORES):
        b, half = core // 2, core % 2
        s0 = half * S_LOC
        out[b, s0 : s0 + S_LOC] = np.asarray(
            res.results[core]["out"], dtype=np.float32
        )
    if _trace:
        return out, res
    return out



# revision 11
# speedup vs baseline: 1.1584x; 1.1162x over previous
"""LocalWindowAttention TRN2 kernel.

Full inputs -> full output. Sharding: 8 cores = batch(4) x seq-half(2).
Each core computes 2048 query positions; k/v halos (128 each side) come
from overlapping the per-core x slice, so no collectives are needed.

Math (per core, matching reference):
  qkv = x @ Wqkv + bqkv  (q scaled by 1/sqrt(1024) via the activation)
  banded attention, window 128, block size 128: query tile e attends key
  tiles e-1, e, e+1 with a static band mask |kpos - qpos| <= 128.
  Softmax without max-subtraction (scores are O(0.1)); invalid keys are
  zeroed AFTER exp via a 0/1 band mask, out-of-sequence keys are zeroed
  via a validity indicator carried as a 65th column of v (which also
  yields the softmax denominator through the PV matmul).
  out = attn_out @ Wout + bout

Dtypes: q/k projection in fp8e4m3 DoubleRow (2x PE rate; weights scaled
x50 on host, descaled in the bias activation; position chunks of 512 so
the matmul stream matches the 256-col LDWEIGHTS time). v/out
projections and attention in fp16. Verified numerically: q/k fp8 adds
~6e-3 rel err (scores are O(0.1) so softmax is insensitive); fp8 on v
would add ~3e-2 and is NOT used.

Scheduling: the attention softmax chain (scores -> EXP -> mask -> PV ->
normalize -> transpose) spans four engines with ~1us of latency per
head-pair but only ~0.8us of PE work, so attention-only stretches leave
the PE sparse enough for the HAM clock gate to re-throttle it to
1.2GHz. The emitter therefore interleaves the dense projection matmul
groups (q/k chunks, v tiles, out-projections) 1:1 between attention
pairs as "filler", keeping the PE dense and warm for the whole kernel.
"""

import sys

import numpy as np

for _p in ("/opt/trn_rl_repo",):
    if _p not in sys.path:
        sys.path.insert(0, _p)

import ml_dtypes  # noqa: E402

import concourse.bass as bass  # noqa: E402,F401
import concourse.mybir as mybir  # noqa: E402
import concourse.tile as tile  # noqa: E402
from concourse import bacc  # noqa: E402
from concourse.bass_utils import run_bass_kernel_spmd  # noqa: E402
from concourse.masks import make_identity  # noqa: E402

F32 = mybir.dt.float32
FP16 = mybir.dt.float16
FP8 = mybir.dt.float8e4
NP_FP8 = ml_dtypes.float8_e4m3

B, S, D = 4, 4096, 1024
H, DH, W = 16, 64, 128
N_CORES = 8
S_LOC = 2048            # query positions per core
T_Q = S_LOC // W        # 16 query tiles per core
T_EXT = T_Q + 2         # 18 extended tiles (with halo)
S_EXT = T_EXT * W       # 2304
NQK = 2 * D             # q+k projected features
KC = D // 128           # 8 contraction chunks
HP = H // 2             # 8 head pairs
VCOL = DH + 1           # 64 v dims + indicator column

SIXTH = S_EXT // 6      # 384 positions per x streaming chunk

WSCALE = 50.0           # fp8 weight prescale (host) -> descale in activation
QK_DESCALE = 1.0 / WSCALE
Q_DESCALE = 1.0 / (WSCALE * np.sqrt(D))

# q/k projection position chunks (512-wide so the DoubleRow matmul
# stream time equals its LDWEIGHTS time; q skips the two halo tiles)
Q_CHUNKS = [(128, 512), (640, 512), (1152, 512), (1664, 512)]
K_CHUNKS = [(0, 512), (512, 512), (1024, 512), (1536, 512), (2048, 256)]


def _build_nc():
    nc = bacc.Bacc(
        "TRN2",
        target_bir_lowering=False,
        debug=False,
        num_devices=N_CORES,
    )

    xT_d = nc.dram_tensor("xT", [D, S_EXT], FP16, kind="ExternalInput").ap()
    xT8_d = nc.dram_tensor("xT8", [D, S_EXT], FP8, kind="ExternalInput").ap()
    wqk8_d = nc.dram_tensor("wqk8", [D, NQK], FP8, kind="ExternalInput").ap()
    wv_d = nc.dram_tensor("wv", [D, D], FP16, kind="ExternalInput").ap()
    bqk_d = nc.dram_tensor("bqk", [128, 16], F32, kind="ExternalInput").ap()
    bvb_d = nc.dram_tensor("bvb", [1, D], FP16, kind="ExternalInput").ap()
    wout_d = nc.dram_tensor("wout", [D, D], FP16, kind="ExternalInput").ap()
    boutb_d = nc.dram_tensor("boutb", [1, D], FP16, kind="ExternalInput").ap()
    trimask_d = nc.dram_tensor("trimask", [128, 3 * W], FP16, kind="ExternalInput").ap()
    indp_d = nc.dram_tensor("indp", [T_EXT, 128], F32, kind="ExternalInput").ap()
    out_d = nc.dram_tensor("out", [S_LOC, D], FP16, kind="ExternalOutput").ap()

    with tile.TileContext(nc) as tc:
        _emit(tc, xT_d, xT8_d, wqk8_d, wv_d, bqk_d, bvb_d, wout_d, boutb_d,
              trimask_d, indp_d, out_d)
    nc.compile()
    return nc


def _emit(tc, xT_d, xT8_d, wqk8_d, wv_d, bqk_d, bvb_d, wout_d, boutb_d,
          trimask_d, indp_d, out_d):
    nc = tc.nc

    with (
        tc.tile_pool(name="consts", bufs=1) as consts,
        tc.tile_pool(name="dram", bufs=1, space="DRAM") as dram,
    ):
        # ---- constants (gpsimd ring; bqk first — the first q/k
        # activation needs it) ----
        bqk_sb = consts.tile([128, 16], F32)
        nc.gpsimd.dma_start(bqk_sb[:], bqk_d[:])
        ind_sb = consts.tile([128, T_EXT], F32)
        nc.gpsimd.dma_start(ind_sb[:], indp_d.rearrange("t p -> p t"))
        # the remaining consts are descriptor-heavy broadcasts that
        # starve the first-needed weight/x8 DMAs if issued up front;
        # they are emitted later via emit_late_consts()
        bvb_sb = consts.tile([128, D], FP16)
        trimask_sb = consts.tile([128, 2, 3, W], FP16)
        boutb_sb = consts.tile([128, D], FP16)
        ident_sb = consts.tile([128, 128], FP16)
        make_identity(nc, ident_sb[:])

        def emit_late_consts():
            nc.gpsimd.dma_start(bvb_sb[:], bvb_d.to_broadcast((128, D)))
            for h2 in range(2):
                nc.gpsimd.dma_start(
                    trimask_sb[:, h2],
                    trimask_d.rearrange("p (c w) -> p c w", c=3),
                )
            nc.gpsimd.dma_start(boutb_sb[:], boutb_d.to_broadcast((128, D)))

        with (
            tc.tile_pool(name="stores", bufs=1) as stores,
            tc.tile_pool(name="p2_e", bufs=4) as p2_e,
            tc.tile_pool(name="p2_sm", bufs=6) as p2_sm,
            tc.tile_pool(name="p2_ao", bufs=4) as p2_ao,
            tc.tile_pool(name="p2_out", bufs=2) as p2_out,
            tc.tile_pool(name="p1_ps", bufs=2, space="PSUM") as p1_ps,
            tc.tile_pool(name="ps_s", bufs=4, space="PSUM") as ps_s_pool,
            tc.tile_pool(name="ps_po", bufs=2, space="PSUM") as ps_po_pool,
            tc.tile_pool(name="p1_w", bufs=1) as p1_w,
            tc.tile_pool(name="p1_x", bufs=2) as p1_x,
        ):
            # q/k in (feature-pair partitions x positions); v in
            # (positions x head x 65) with the indicator column
            # q skips the two halo tiles: columns are positions 128..2176
            # of the extended axis, stored at offset-128
            q_store = stores.tile([128, KC, S_LOC], FP16)
            k_store = stores.tile([128, KC, S_EXT], FP16)
            v_store = stores.tile([128, T_EXT, H, VCOL], FP16)
            # fp8 x, fully resident (18KB/partition) so q/k matmul
            # position chunks decouple from the DMA stream
            x8_store = stores.tile([128, KC, S_EXT], FP8)

            wout_sb = p1_w.tile([128, KC, D], FP16)
            w8_sb = p1_w.tile([128, KC, NQK], FP8)
            wv_sb = p1_w.tile([128, KC, D], FP16)

            def emit_x8_dma(sixth):
                tp0 = sixth * SIXTH
                for kc in range(KC):
                    nc.scalar.dma_start(
                        x8_store[:, kc, tp0 : tp0 + SIXTH],
                        xT8_d[kc * 128 : (kc + 1) * 128, tp0 : tp0 + SIXTH],
                    )

            # weights on the sync ring: w8 piece 0 first (first q/k
            # matmuls), wv interleaved early (v tiles need both pieces),
            # wout last (first out-projection is far in)
            w8_piece = lambda p: [
                nc.sync.dma_start(
                    w8_sb[:, kc, p * 512 : (p + 1) * 512],
                    wqk8_d[kc * 128 : (kc + 1) * 128, p * 512 : (p + 1) * 512],
                )
                for kc in range(KC)
            ]
            wv_piece = lambda p: [
                nc.sync.dma_start(
                    wv_sb[:, kc, p * 512 : (p + 1) * 512],
                    wv_d[kc * 128 : (kc + 1) * 128, p * 512 : (p + 1) * 512],
                )
                for kc in range(KC)
            ]
            w8_piece(0)
            wv_piece(0)
            w8_piece(1)
            wv_piece(1)
            w8_piece(2)
            w8_piece(3)
            for kc in range(KC):
                nc.sync.dma_start(wout_sb[:, kc, :], wout_d[kc * 128 : (kc + 1) * 128, :])

            def qk_item(ci, m):
                """One q/k projection matmul group: output feature chunk
                m over position chunk ci (fp8 DoubleRow, 4 accumulating
                matmuls + bias/descale activation). ~0.9us of dense PE."""
                if m < KC:
                    o0, on = Q_CHUNKS[ci]
                    dst, mm = q_store, m
                else:
                    o0, on = K_CHUNKS[ci]
                    dst, mm = k_store, m - KC
                ps = p1_ps.tile([128, 512], F32, tag="ps1", name="ps")
                for kcp in range(KC // 2):
                    nc.tensor.matmul(
                        ps[:, :on],
                        lhsT=w8_sb[:, 2 * kcp : 2 * kcp + 2, m * 128 : (m + 1) * 128],
                        rhs=x8_store[:, 2 * kcp : 2 * kcp + 2, o0 : o0 + on],
                        start=(kcp == 0),
                        stop=(kcp == KC // 2 - 1),
                        perf_mode=mybir.MatmulPerfMode.DoubleRow,
                    )
                d0 = o0 - 128 if m < KC else o0  # q_store is offset by the halo
                nc.scalar.activation(
                    dst[:, mm, d0 : d0 + on],
                    ps[:, :on],
                    mybir.ActivationFunctionType.Identity,
                    bias=bqk_sb[:, m : m + 1],
                    scale=Q_DESCALE if m < KC else QK_DESCALE,
                )

            def qk_chunk_items(ci):
                ms = range(16) if ci < len(Q_CHUNKS) else range(KC, 16)
                return [(lambda ci=ci, m=m: qk_item(ci, m)) for m in ms]

            # per-sixth xT staging for the v projection (pool, 3 bufs)
            xT_tiles = {}

            def v_dma_item(sixth):
                tp0 = sixth * SIXTH
                xT_sb = p1_x.tile([128, KC, SIXTH], FP16, tag="xt", name="xT_sb")
                xT_tiles[sixth] = xT_sb
                for kc in range(KC):
                    nc.scalar.dma_start(
                        xT_sb[:, kc, :],
                        xT_d[kc * 128 : (kc + 1) * 128, tp0 : tp0 + SIXTH],
                    )

            def v_item(t, g):
                """Half of one v tile: 8 accumulating N=512 matmuls +
                bias add. ~1.7us of dense PE. g==1 also finalizes the
                tile (indicator column; boundary-tile zeroing)."""
                sixth, tt = t // 3, t % 3
                xT_sb = xT_tiles[sixth]
                ps = p1_ps.tile([128, 512], F32, tag="ps1", name="ps")
                for kc in range(KC):
                    nc.tensor.matmul(
                        ps[:],
                        lhsT=xT_sb[:, kc, tt * 128 : (tt + 1) * 128],
                        rhs=wv_sb[:, kc, g * 512 : (g + 1) * 512],
                        start=(kc == 0),
                        stop=(kc == KC - 1),
                    )
                nc.vector.tensor_add(
                    v_store[:, t, g * 8 : (g + 1) * 8, 0:DH],
                    ps.rearrange("p (h d) -> p h d", d=DH),
                    bvb_sb[:, g * 512 : (g + 1) * 512].rearrange(
                        "p (h d) -> p h d", d=DH
                    ),
                )
                if g == 1 and t in (0, T_EXT - 1):
                    # only the two halo tiles can be out-of-sequence
                    # (the indicator column itself is set by one
                    # upfront memset over all tiles)
                    nc.vector.tensor_scalar_mul(
                        v_store[:, t], v_store[:, t], ind_sb[:, t : t + 1]
                    )

            def v_sixth_items(sixth):
                items = [lambda sixth=sixth: v_dma_item(sixth)]
                for tt in range(3):
                    t = sixth * 3 + tt
                    for g in range(2):
                        items.append(lambda t=t, g=g: v_item(t, g))
                return items

            def out_proj_item(e, g, aoT_sb):
                """Out-projection half for query tile e: 8 accumulating
                N=512 matmuls + bias + store. ~1.8us of dense PE."""
                ps_f = p1_ps.tile([128, 512], F32, tag="ps1", name="ps_f")
                for a in range(HP):
                    nc.tensor.matmul(
                        ps_f[:],
                        lhsT=aoT_sb[:, a, :],
                        rhs=wout_sb[:, a, g * 512 : (g + 1) * 512],
                        start=(a == 0),
                        stop=(a == HP - 1),
                    )
                fo = p2_out.tile([128, 512], FP16, tag="fo", name="fo")
                nc.vector.tensor_add(
                    fo[:], ps_f[:], boutb_sb[:, g * 512 : (g + 1) * 512]
                )
                eng = nc.sync if g == 0 else nc.gpsimd
                eng.dma_start(
                    out_d[(e - 1) * 128 : e * 128, g * 512 : (g + 1) * 512],
                    fo[:],
                )

            # aoT copies are emitted one pair late (after the next
            # pair's EXPs) so the copy's wait-on-transpose never blocks
            # the scalar queue in front of an EXP
            pending_copy = []

            def flush_copies():
                while pending_copy:
                    pending_copy.pop(0)()

            def emit_pair(e, a, aoT_sb):
                """Softmax chain for head-pair a of query tile e."""
                # scoresT per head, one 1-bank PSUM tile per head so the
                # EXP/mask/PV chain for h2=0 starts while h2=1's scores
                # finish; c-outer/h2-inner alternates PE row groups
                # (tile_position (0,0)/(64,0)) between adjacent matmuls
                # so LDWEIGHTS can pull ahead and the two 64-contraction
                # matmuls can overlap in the array
                ps_h = [
                    ps_s_pool.tile([128, 3, W], F32, tag="ps_s", name="ps_s")
                    for _ in range(2)
                ]
                for c in range(3):
                    t = e - 1 + c
                    for h2 in range(2):
                        pr = slice(64 * h2, 64 * h2 + 64)
                        nc.tensor.matmul(
                            ps_h[h2][:, c, :],
                            lhsT=k_store[pr, a, t * 128 : (t + 1) * 128],
                            rhs=q_store[pr, a, (e - 1) * 128 : e * 128],
                            start=True,
                            stop=True,
                        )
                e_sb = p2_e.tile([128, 2, 3, W], FP16, name="e_sb")
                for h2 in range(2):
                    nc.scalar.activation(
                        e_sb[:, h2],
                        ps_h[h2][:],
                        mybir.ActivationFunctionType.Exp,
                    )
                flush_copies()
                # band mask: only the two triangular chunks (c=1 is
                # fully in-band); DVE — GpSimd is 3.5x slower and was
                # sitting on the critical path
                for h2 in range(2):
                    nc.vector.tensor_mul(
                        e_sb[:, h2, 0:3:2, :],
                        e_sb[:, h2, 0:3:2, :],
                        trimask_sb[:, h2, 0:3:2, :],
                    )

                # both heads' PV outputs share ONE PSUM bank
                # ([128, 2, 65] = 520B)
                no_sb = p2_sm.tile([128, 2, DH], FP16, tag="no", name="no_sb")
                ps_o = ps_po_pool.tile([128, 2 * VCOL], F32, tag="po", name="ps_o")
                ps_o2 = ps_o.rearrange("p (a v) -> p a v", a=2)
                for h2 in range(2):
                    for c in range(3):
                        nc.tensor.matmul(
                            ps_o2[:, h2, :],
                            lhsT=e_sb[:, h2, c, :],
                            rhs=v_store[:, e - 1 + c, 2 * a + h2, :],
                            start=(c == 0),
                            stop=(c == 2),
                        )
                rcp = p2_sm.tile([128, 2], F32, tag="rcp", name="rcp")
                nc.vector.reciprocal(rcp[:], ps_o2[:, :, DH])
                for h2 in range(2):
                    nc.vector.tensor_scalar_mul(
                        no_sb[:, h2, :], ps_o2[:, h2, 0:DH], rcp[:, h2 : h2 + 1]
                    )
                ps_t = ps_po_pool.tile([128, 2 * VCOL], F32, tag="po", name="ps_t")
                ps_t16 = ps_t.bitcast(FP16)[:, :128]
                nc.tensor.transpose(
                    ps_t16[:], no_sb.rearrange("p a d -> p (a d)"), ident_sb[:]
                )

                def emit_copy(a=a, ps_t16=ps_t16):
                    if a % 2 == 0:
                        nc.scalar.copy(aoT_sb[:, a, :], ps_t16[:])
                    else:
                        nc.vector.tensor_copy(aoT_sb[:, a, :], ps_t16[:])

                pending_copy.append(emit_copy)

            # the v indicator column is constant 1.0 across all tiles
            # (the projection writes cols 0:DH only — disjoint), so one
            # upfront memset covers the whole kernel
            nc.vector.memset(v_store[:, :, :, DH : DH + 1], 1.0)

            # phase 0: projections only (nothing to interleave yet)
            emit_x8_dma(0)
            emit_x8_dma(1)
            for it in qk_chunk_items(0):
                it()
            emit_late_consts()
            emit_x8_dma(2)
            # interleave the first v tiles into the second q/k chunk so
            # the v pipeline starts as soon as wv/xT land
            f = v_sixth_items(0) + v_sixth_items(1)
            for it in qk_chunk_items(1):
                it()
                if f:
                    f.pop(0)()
            emit_x8_dma(3)
            emit_x8_dma(4)
            for it in f:
                it()

            # ---- interleaved emission: uniform dense-work pacing.
            # The remaining projection items (deadline-bearing, proj_q)
            # and the out-projections (freely deferrable, out_q; each
            # tile's out-proj is enqueued two tiles after its attention)
            # are drained at a constant rate across all 128 pair slots
            # so the PE never goes sparse enough for the HAM clock gate
            # to re-throttle it — previously the fillers ran dry around
            # tile 11 and the tail ran at 1.2GHz two-thirds of the time.
            # v items lead their qk phase-mates: their deadlines (the
            # attention tiles that read v_store) are tighter. All proj
            # deadlines are met at a drain pace of 68 items over the
            # first ~100 pair slots (checked against each refill's
            # first-use tile with >=3 slots of margin).
            proj_q = v_sixth_items(2) + qk_chunk_items(2)
            out_q = []
            out_emitted = 0
            ao_live = {}
            acc_p = 0.0
            acc_o = 0.0
            pace_p = 68.0 / 100.0
            pace_o = 2.0 / HP  # matches the enqueue rate: backlog <= 1 tile
            for e in range(1, T_Q + 1):
                if e == 4:
                    emit_x8_dma(5)
                    proj_q += v_sixth_items(3) + qk_chunk_items(3)
                elif e == 7:
                    proj_q += v_sixth_items(4) + qk_chunk_items(4)
                elif e == 9:
                    proj_q += v_sixth_items(5)
                eo = e - 2
                if eo in ao_live:
                    aoT_prev = ao_live.pop(eo)
                    out_q.append(
                        lambda eo=eo, t=aoT_prev: out_proj_item(eo, 0, t)
                    )
                    out_q.append(
                        lambda eo=eo, t=aoT_prev: out_proj_item(eo, 1, t)
                    )
                # the p2_ao buf reused by tile e held tile e-4's aoT: its
                # out-projection must be EMITTED before this allocation
                # or the pool dependency cycles (deadlock in scheduling)
                while out_emitted < 2 * max(0, e - 4) and out_q:
                    out_q.pop(0)()
                    out_emitted += 1
                aoT_sb = p2_ao.tile([128, HP, 128], FP16, name="aoT")
                ao_live[e] = aoT_sb
                for a in range(HP):
                    emit_pair(e, a, aoT_sb)
                    acc_p += pace_p
                    while acc_p >= 1.0 and proj_q:
                        acc_p -= 1.0
                        proj_q.pop(0)()
                    acc_o += pace_o
                    while acc_o >= 1.0 and out_q:
                        acc_o -= 1.0
                        out_q.pop(0)()
                        out_emitted += 1
            flush_copies()
            for it in proj_q + out_q:
                it()
            for eo in sorted(ao_live):
                out_proj_item(eo, 0, ao_live[eo])
                out_proj_item(eo, 1, ao_live[eo])


_NC_CACHE = None


def _get_nc():
    global _NC_CACHE
    if _NC_CACHE is None:
        _NC_CACHE = _build_nc()
    return _NC_CACHE


def _host_inputs(x, Wqkv, bqkv, Wout, bout):
    """Build the 8 per-core input maps."""
    x = np.asarray(x, dtype=np.float32)
    Wqkv = np.asarray(Wqkv, dtype=np.float32)
    bqkv = np.asarray(bqkv, dtype=np.float32)
    Wout = np.asarray(Wout, dtype=np.float32)
    bout = np.asarray(bout, dtype=np.float32)

    scale = np.float32(1.0 / np.sqrt(D))
    bs = bqkv.copy()
    bs[:D] *= scale

    # q/k weights: prescale x50 for fp8 range (descaled in the activation)
    wqk8 = np.clip(Wqkv[:, :NQK] * WSCALE, -240.0, 240.0).astype(NP_FP8)
    wv16 = Wqkv[:, NQK:].astype(np.float16)

    bqk = np.ascontiguousarray(bs[:NQK].reshape(16, 128).T)  # (128, 16)
    bvb = bs[NQK:].reshape(1, D)
    boutb = bout.reshape(1, D)

    # band mask in (j_within_chunk, chunk, i) layout flattened to (128, 384)
    jc = np.arange(128)[:, None]
    i = np.arange(128)[None, :]
    tm = np.ones((128, 3, 128), dtype=np.float32)
    tm[:, 0] = (jc >= i).astype(np.float32)
    tm[:, 2] = (jc <= i).astype(np.float32)
    trimask = tm.reshape(128, 3 * W).astype(np.float16)

    in_maps = []
    for core in range(N_CORES):
        b, half = core // 2, core % 2
        s0 = half * S_LOC
        lo, hi = s0 - W, s0 + S_LOC + W
        xp = np.zeros((S_EXT, D), dtype=np.float32)
        src_lo, src_hi = max(lo, 0), min(hi, S)
        xp[src_lo - lo : src_hi - lo] = x[b, src_lo:src_hi]
        xpT = np.ascontiguousarray(xp.T)
        xT = xpT.astype(np.float16)
        xT8 = np.clip(xpT, -240.0, 240.0).astype(NP_FP8)

        valid = np.ones(S_EXT, dtype=np.float32)
        if lo < 0:
            valid[: -lo] = 0.0
        if hi > S:
            valid[S - hi :] = 0.0
        indp = np.ascontiguousarray(valid.reshape(T_EXT, 128))

        in_maps.append(
            {
                "xT": xT,
                "xT8": xT8,
                "wqk8": wqk8,
                "wv": wv16,
                "bqk": bqk,
                "bvb": bvb.astype(np.float16),
                "wout": Wout.astype(np.float16),
                "boutb": boutb.astype(np.float16),
                "trimask": trimask,
                "indp": indp,
            }
        )
    return in_maps


def kernel(x, Wqkv, bqkv, Wout, bout, _trace=False, _trace_cores=None):
    in_maps = _host_inputs(x, Wqkv, bqkv, Wout, bout)
    nc = _get_nc()
    res = run_bass_kernel_spmd(
        nc,
        in_maps,
        list(range(N_CORES)),
        trace=_trace,
        trace_cores=_trace_cores,
    )
    out = np.empty((B, S, D), dtype=np.float32)
    for core in range(N_CORES):
        b, half = core // 2, core % 2
        s0 = half * S_LOC
        out[b, s0 : s0 + S_LOC] = np.asarray(
            res.results[core]["out"], dtype=np.float32
        )
    if _trace:
        return out, res
    return out

